# revision 1
# baseline (speedup 1.0000x reference)
"""Trainium2 Bass kernel for nn_MultiHeadAttention_47382079209593.

Full-input contract: kernel(**inputs) takes the complete unsharded tensors and
returns the full (out, decomposed) pair, distributing work across 8 NeuronCores
internally.

Sharding:
  - Attention (qkv proj, softmax, out proj): data-parallel over batch, 8
    batches per core.
  - decomposed = (out[:, -1, :] @ W_ctx): column-parallel over W_ctx's
    512*512 output dim -> core i owns block positions w in [64i, 64i+64) for
    ALL 64 batches.  The 64x512 last-token activations are shared via an
    on-device AllGather (16 KB per core).
  - decomposed2 = (prev + dec) @ W_proj2: row-parallel over the (b, w) dim,
    no communication needed.

All heavy matmuls run in float32r (full-rate fp32 PE mode, ~1.6e-4 rel err).
"""

import sys

if '/opt/trn_rl_repo' not in sys.path:
    sys.path.insert(0, '/opt/trn_rl_repo')

import numpy as np

import concourse.bass as bass
import concourse.tile as tile
from concourse import bacc, mybir
from concourse.bass_utils import run_bass_kernel_spmd

F32 = mybir.dt.float32
BF16 = mybir.dt.bfloat16
F32R = mybir.dt.float32r
EXP = mybir.ActivationFunctionType.Exp

B, W, C = 64, 512, 512
H = 8
DH = C // H          # 64
BLOCK = 512
N_CORES = 8
BPC = B // N_CORES   # 8 batches per core
WPC = BLOCK // N_CORES  # 64 block positions per core



def r(ap):
    return ap.bitcast(F32R)


def build_kernel():
    nc = bacc.Bacc("TRN2", num_devices=N_CORES)

    x_ext = nc.dram_tensor("x", [BPC, W, C], F32, kind="ExternalInput")
    prev_ext = nc.dram_tensor("prev", [B, WPC, C], F32, kind="ExternalInput")
    wattn_ext = nc.dram_tensor("w_attn", [C, 3 * C], F32, kind="ExternalInput")
    wctx_ext = nc.dram_tensor("w_ctx", [C, WPC * C], BF16, kind="ExternalInput")
    wproj_ext = nc.dram_tensor("w_proj", [C, C], F32, kind="ExternalInput")
    wproj2_ext = nc.dram_tensor("w_proj2", [C, C], F32, kind="ExternalInput")
    ident_ext = nc.dram_tensor("ident", [128, 128], F32, kind="ExternalInput")
    maskt_ext = nc.dram_tensor("maskt", [128, 128], F32, kind="ExternalInput")  # 0/1 keep-mask
    ones_ext = nc.dram_tensor("ones", [128, 1], F32, kind="ExternalInput")

    out_ext = nc.dram_tensor("out", [BPC, W, C], F32, kind="ExternalOutput")
    dec_ext = nc.dram_tensor("dec", [B, WPC, C], F32, kind="ExternalOutput")

    cc_in = nc.dram_tensor("cc_in", [BPC, C], F32)
    cc_out = nc.dram_tensor("cc_out", [B, C], F32, addr_space="Shared")

    from contextlib import ExitStack

    with tile.TileContext(nc) as tc, ExitStack() as ctx:
        if True:
            pool = lambda name, bufs, **kw: ctx.enter_context(
                tc.tile_pool(name=name, bufs=bufs, **kw))
            consts = pool("consts", 1)
            weights = pool("weights", 1)
            persist = pool("persist", 1)
            # PSUM pools: 8 banks total
            ps_mm = pool("ps_mm", 3, space="PSUM")
            ps_sc = pool("ps_sc", 3, space="PSUM")
            ps_ot = pool("ps_ot", 2, space="PSUM")
            p_x = pool("p_x", 1)
            p_xt = pool("p_xt", 1)
            p_qkt = pool("p_qkt", 1)
            p_v = pool("p_v", 2)
            p_exp = pool("p_exp", 3)
            p_out = pool("p_out", 2)
            p_small = pool("p_small", 2)
            p_cp = pool("p_cp", 2)
            p_wc = pool("p_wc", 2)
            p_dec = pool("p_dec", 2)

            # ---- constants & weights ----
            ident = consts.tile([128, 128], F32)
            nc.sync.dma_start(out=ident[:], in_=ident_ext[:])
            maskt = consts.tile([128, 128], F32)
            nc.sync.dma_start(out=maskt[:], in_=maskt_ext[:])
            ones = consts.tile([128, 1], F32)
            nc.sync.dma_start(out=r(ones[:]), in_=r(ones_ext[:]))

            wattn = weights.tile([128, 4, 3 * C], F32)
            nc.sync.dma_start(
                out=r(wattn[:]),
                in_=r(wattn_ext[:].rearrange("(k p) c -> p k c", p=128)))
            wproj = weights.tile([64, H, C], F32)
            nc.sync.dma_start(
                out=r(wproj[:]),
                in_=r(wproj_ext[:].rearrange("(h p) c -> p h c", p=64)))
            wproj2 = weights.tile([128, 4, C], F32)
            nc.sync.dma_start(
                out=r(wproj2[:]),
                in_=r(wproj2_ext[:].rearrange("(k p) c -> p k c", p=128)))

            lastT = persist.tile([64, H], F32)  # staging of out_last^T per batch

            # ================= attention phase (per local batch) ============
            for b in range(BPC):
                # load x_b [4 tok-chunks, 128, 512]
                x_sb = p_x.tile([128, 4, C], F32)
                nc.sync.dma_start(
                    out=x_sb[:],
                    in_=x_ext[b].rearrange("(t p) c -> p t c", p=128))
                # transpose -> xT [128, cc, tok]
                xt_sb = p_xt.tile([128, 4, W], F32)
                for cc in range(4):
                    xp = ps_sc.tile([128, W], F32, tag="sc")
                    for t in range(4):
                        nc.tensor.transpose(
                            xp[:, t * 128:(t + 1) * 128],
                            x_sb[:, t, cc * 128:(cc + 1) * 128], ident[:])
                    nc.vector.tensor_copy(r(xt_sb[:, cc, :]), xp[:])

                # qkT [128, mc(8), tok] and v interleaved so head-0
                # operands (mc 0/4, v chunk 0) are ready earliest
                qkt = p_qkt.tile([128, 8, W], F32)
                v_sb = p_v.tile([128, 4, H, 65], F32)

                def qk_group(mc):
                    ps = ps_mm.tile([128, W], F32, tag="mm")
                    for kc in range(4):
                        nc.tensor.matmul(
                            ps[:],
                            r(wattn[:, kc, mc * 128:(mc + 1) * 128]),
                            r(xt_sb[:, kc, :]),
                            start=(kc == 0), stop=(kc == 3))
                    nc.vector.tensor_copy(r(qkt[:, mc, :]), ps[:])

                def v_group(t):
                    ps = ps_mm.tile([128, C], F32, tag="mm")
                    for kc in range(4):
                        nc.tensor.matmul(
                            ps[:],
                            r(xt_sb[:, kc, t * 128:(t + 1) * 128]),
                            r(wattn[:, kc, 2 * C:3 * C]),
                            start=(kc == 0), stop=(kc == 3))
                    nc.vector.tensor_copy(
                        r(v_sb[:, t, :, 0:64]),
                        ps[:].rearrange("p (h d) -> p h d", h=H))
                    nc.vector.memset(v_sb[:, t, :, 64], 1.0)

                qk_group(0); qk_group(4); v_group(0)
                qk_group(1); qk_group(5); v_group(1)
                qk_group(2); qk_group(6); v_group(2)
                qk_group(3); qk_group(7); v_group(3)

                outt = p_out.tile([64, H, W], F32)  # normalized outT per head

                def make_head(h, et, ot):
                    base = (h % 2) * 64
                    qt = qkt[base:base + 64, h // 2, :]
                    kt = qkt[base:base + 64, 4 + h // 2, :]

                    def scores_strip(ki):
                        n = W - ki * 128
                        sc = ps_sc.tile([128, W], F32, tag="sc")
                        nc.tensor.matmul(
                            sc[:, :n],
                            r(kt[:, ki * 128:(ki + 1) * 128]),
                            r(qt[:, ki * 128:]),
                            start=True, stop=True)
                        nc.scalar.activation(
                            r(et[:, ki, :n]), sc[:, :n], EXP, scale=0.125)
                        # causal 0/1 mask on the diagonal block
                        nc.vector.tensor_mul(
                            r(et[:, ki, :128]), r(et[:, ki, :128]), maskt[:])

                    def attnv_strip(ki):
                        n = W - ki * 128
                        nc.tensor.matmul(
                            ot[0:65, ki * 128:],
                            r(v_sb[:, ki, h, :]),
                            r(et[:, ki, :n]),
                            start=(ki == 0), stop=(ki == 3))

                    def finish():
                        recip = p_small.tile([1, W], F32)
                        nc.vector.reciprocal(recip[:], ot[64:65, :])
                        bcast = p_small.tile([64, W], F32)
                        nc.gpsimd.partition_broadcast(bcast[:], recip[:])
                        nc.vector.tensor_mul(
                            r(outt[:, h, :]), ot[0:64, :], bcast[:])

                    return scores_strip, attnv_strip, finish

                # heads in pairs: even head uses partitions 0-63 (PE rows
                # 0-63), odd head rows 64-127 -> score matmuls of the pair
                # land on disjoint PE row groups and can overlap.
                for hp in range(4):
                    h0, h1 = 2 * hp, 2 * hp + 1
                    et0 = p_exp.tile([128, 4, W], F32, tag="et")
                    ot0 = ps_ot.tile([65, W], F32, tag="ot")
                    et1 = p_exp.tile([128, 4, W], F32, tag="et")
                    ot1 = ps_ot.tile([65, W], F32, tag="ot")
                    s0, a0, f0 = make_head(h0, et0, ot0)
                    s1, a1, f1 = make_head(h1, et1, ot1)
                    s0(0); s1(0)
                    s0(1); a0(0)
                    s1(1); a1(0)
                    s0(2); a0(1)
                    s1(2); a1(1)
                    s0(3); a0(2)
                    s1(3); a1(2)
                    a0(3); a1(3)
                    f0(); f1()

                # stage out_last^T columns: lastT[d, h] = outT[d, h, 511]
                nc.vector.tensor_copy(lastT[:, :], outt[:, :, W - 1])

                # out proj: out[tok, :] = sum_h outT[:, h, tok].T @ Wproj[h]
                pr = p_cp.tile([128, 4, C], F32)
                for t in range(4):
                    ps = ps_mm.tile([128, C], F32, tag="mm")
                    for h in range(H):
                        nc.tensor.matmul(
                            ps[:],
                            r(outt[:, h, t * 128:(t + 1) * 128]),
                            r(wproj[:, h, :]),
                            start=(h == 0), stop=(h == 7))
                    nc.vector.tensor_copy(pr[:, t, :], ps[:])
                nc.sync.dma_start(
                    out=out_ext[b].rearrange("(t p) c -> p t c", p=128),
                    in_=pr[:])

                # out_last natural row for this batch -> cc_in[b, h*64+d]
                cc_ap = cc_in[:]
                nc.sync.dma_start(
                    out=bass.AP(tensor=cc_ap.tensor, offset=b * C,
                                ap=[[1, 64], [64, H]]),
                    in_=lastT[:, :])

            # ================= collective =================
            nc.gpsimd.collective_compute(
                "AllGather",
                mybir.AluOpType.bypass,
                ins=[cc_in[:]],
                outs=[cc_out[:]],
                replica_groups=[list(range(N_CORES))],
            )

            # ================= decomposed phase =================
            ol = p_dec.tile([64, C], F32)  # out_last [64 batches, 512]
            nc.sync.dma_start(out=ol[:], in_=cc_out[:])
            lastT_all = persist.tile([128, 4, 64], BF16)
            for t in range(4):
                xp = ps_sc.tile([128, 64], F32, tag="sc")
                nc.tensor.transpose(
                    xp[:], ol[:, t * 128:(t + 1) * 128], ident[0:64, 0:64])
                nc.vector.tensor_copy(lastT_all[:, t, :], xp[:])

            for w in range(WPC):
                wc = p_wc.tile([128, 4, C], BF16)
                nc.scalar.dma_start(
                    out=wc[:],
                    in_=wctx_ext[:, w * C:(w + 1) * C].rearrange(
                        "(k p) c -> p k c", p=128))
                dps = ps_mm.tile([64, C], F32, tag="mm")
                for kc in range(4):
                    nc.tensor.matmul(
                        dps[0:64, :], lastT_all[:, kc, :], wc[:, kc, :],
                        start=(kc == 0), stop=(kc == 3))
                pv = p_dec.tile([64, C], F32)
                nc.sync.dma_start(out=pv[:], in_=prev_ext[:, w, :])
                s_sb = p_dec.tile([64, C], F32)
                nc.vector.tensor_add(s_sb[:], dps[0:64, :], pv[:])
                st = p_dec.tile([128, 4, 64], F32)
                xp = ps_sc.tile([128, 256], F32, tag="sc")
                for t in range(4):
                    nc.tensor.transpose(
                        xp[:, t * 64:(t + 1) * 64],
                        s_sb[:, t * 128:(t + 1) * 128],
                        ident[0:64, 0:64])
                nc.scalar.copy(r(st[:]), xp[:])
                d2 = ps_sc.tile([64, C], F32, tag="sc")
                for t in range(4):
                    nc.tensor.matmul(
                        d2[0:64, :], r(st[:, t, :]), r(wproj2[:, t, :]),
                        start=(t == 0), stop=(t == 3))
                d2s = p_dec.tile([64, C], F32)
                nc.scalar.copy(d2s[:], d2[0:64, :])
                nc.sync.dma_start(out=dec_ext[:, w, :], in_=d2s[:])

    nc.finalize()
    return nc


_NC_CACHE = None


def _get_nc():
    global _NC_CACHE
    if _NC_CACHE is None:
        _NC_CACHE = build_kernel()
    return _NC_CACHE


def make_in_maps(x, prev_decomposed, W_attn, W_ctx, W_proj, W_proj2):
    import ml_dtypes
    W_ctx = np.asarray(W_ctx).astype(ml_dtypes.bfloat16)
    ident = np.eye(128, dtype=np.float32)
    # scoresT layout [k, q]: keep k <= q within the diagonal block
    kk, qq = np.meshgrid(np.arange(128), np.arange(128), indexing="ij")
    maskt = np.where(kk > qq, np.float32(0.0), np.float32(1.0))
    ones = np.ones((128, 1), dtype=np.float32)

    in_maps = []
    for i in range(N_CORES):
        in_maps.append({
            "x": np.ascontiguousarray(x[i * BPC:(i + 1) * BPC]),
            "prev": np.ascontiguousarray(
                prev_decomposed[:, i * WPC:(i + 1) * WPC, :]),
            "w_attn": np.ascontiguousarray(W_attn),
            "w_ctx": np.ascontiguousarray(
                W_ctx[:, i * WPC * C:(i + 1) * WPC * C]),
            "w_proj": np.ascontiguousarray(W_proj),
            "w_proj2": np.ascontiguousarray(W_proj2),
            "ident": ident,
            "maskt": maskt,
            "ones": ones,
        })
    return in_maps


def run(x, prev_decomposed, W_attn, W_ctx, W_proj, W_proj2, **spmd_kwargs):
    nc = _get_nc()
    in_maps = make_in_maps(x, prev_decomposed, W_attn, W_ctx, W_proj, W_proj2)
    res = run_bass_kernel_spmd(nc, in_maps, list(range(N_CORES)), **spmd_kwargs)
    results = res.results
    out = np.concatenate(
        [np.asarray(results[i]["out"]) for i in range(N_CORES)], axis=0)
    dec = np.concatenate(
        [np.asarray(results[i]["dec"]) for i in range(N_CORES)], axis=1)
    return (out, dec), res


def kernel(x, prev_decomposed, W_attn, W_ctx, W_proj, W_proj2):
    (out, dec), _ = run(
        np.asarray(x, dtype=np.float32),
        np.asarray(prev_decomposed, dtype=np.float32),
        np.asarray(W_attn, dtype=np.float32),
        np.asarray(W_ctx, dtype=np.float32),
        np.asarray(W_proj, dtype=np.float32),
        np.asarray(W_proj2, dtype=np.float32))
    return (out, dec)



# revision 2
# speedup vs baseline: 1.8301x; 1.8301x over previous
"""Trainium2 Bass kernel for nn_MultiHeadAttention_47382079209593.

Full-input contract: kernel(**inputs) takes the complete unsharded tensors and
returns the full (out, decomposed) pair, distributing work across 8 NeuronCores
internally.

Sharding:
  - Attention (qkv proj, softmax, out proj): data-parallel over batch, 8
    batches per core.
  - decomposed = (out[:, -1, :] @ W_ctx): column-parallel over W_ctx's
    512*512 output dim -> core i owns block positions w in [64i, 64i+64) for
    ALL 64 batches.  The 64x512 last-token activations are shared via an
    on-device AllGather (16 KB per core).
  - decomposed2 = (prev + dec) @ W_proj2: row-parallel over the (b, w) dim,
    no communication needed.

All heavy matmuls run in float32r (full-rate fp32 PE mode, ~1.6e-4 rel err).

I/O packing: the PJRT-over-axon dispatch costs ~7 ms per executable argument
(measured: per-arg, not per-byte), so all 8 logical inputs are packed into a
single 1-D f32 blob per core and both outputs into a single 1-D f32 blob.
Offsets below are in f32 words; W_ctx is stored as bf16 bitcast into the f32
blob.
"""

import sys

if '/opt/trn_rl_repo' not in sys.path:
    sys.path.insert(0, '/opt/trn_rl_repo')

import numpy as np

import concourse.bass as bass
import concourse.tile as tile
from concourse import bacc, mybir
from concourse.bass_utils import run_bass_kernel_spmd

F32 = mybir.dt.float32
BF16 = mybir.dt.bfloat16
F32R = mybir.dt.float32r
EXP = mybir.ActivationFunctionType.Exp

B, W, C = 64, 512, 512
H = 8
DH = C // H          # 64
BLOCK = 512
N_CORES = 8
BPC = B // N_CORES   # 8 batches per core
WPC = BLOCK // N_CORES  # 64 block positions per core

# ---- packed input blob layout (f32 words, per core) ----
OX = 0                                  # x shard      [BPC, W, C]
OPREV = OX + BPC * W * C                # prev shard   [B, WPC, C]
OWA = OPREV + B * WPC * C               # W_attn       [C, 3C]
OWP = OWA + C * 3 * C                   # W_proj       [C, C]
OWP2 = OWP + C * C                      # W_proj2      [C, C]
OID = OWP2 + C * C                      # identity     [128, 128]
OMK = OID + 128 * 128                   # causal maskT [128, 128]
OWC = OMK + 128 * 128                   # W_ctx shard  [C, WPC*C] bf16
IN_WORDS = OWC + C * WPC * C // 2
# ---- packed output blob layout (f32 words, per core) ----
OOUT = 0                                # out shard    [BPC, W, C]
ODEC = OOUT + BPC * W * C               # dec shard    [B, WPC, C]
OUT_WORDS = ODEC + B * WPC * C


def r(ap):
    return ap.bitcast(F32R)


def build_kernel():
    nc = bacc.Bacc("TRN2", num_devices=N_CORES)

    io_in = nc.dram_tensor("io_in", [IN_WORDS], F32, kind="ExternalInput")
    io_out = nc.dram_tensor("io_out", [OUT_WORDS], F32, kind="ExternalOutput")

    x_all = io_in[OX:OPREV].rearrange("(b t p c) -> b p t c",
                                      b=BPC, t=4, p=128, c=C)
    prev_all = io_in[OPREV:OWA].rearrange("(b w c) -> b w c", b=B, w=WPC, c=C)
    wattn_src = io_in[OWA:OWP].rearrange("(k p c) -> p k c", k=4, p=128)
    wproj_src = io_in[OWP:OWP2].rearrange("(h p c) -> p h c", h=H, p=64)
    wproj2_src = io_in[OWP2:OID].rearrange("(k p c) -> p k c", k=4, p=128)
    ident_src = io_in[OID:OMK].rearrange("(p c) -> p c", p=128)
    maskt_src = io_in[OMK:OWC].rearrange("(p c) -> p c", p=128)
    wctx_all = io_in[OWC:IN_WORDS].bitcast(BF16).rearrange(
        "(k p m) -> p k m", k=4, p=128, m=WPC * C)

    out_all = io_out[OOUT:ODEC].rearrange("(b t p c) -> b p t c",
                                          b=BPC, t=4, p=128, c=C)
    dec_all = io_out[ODEC:OUT_WORDS].rearrange("(b w c) -> b w c",
                                               b=B, w=WPC, c=C)

    cc_in = nc.dram_tensor("cc_in", [BPC, C], F32)
    cc_out = nc.dram_tensor("cc_out", [B, C], F32, addr_space="Shared")

    from contextlib import ExitStack

    with tile.TileContext(nc) as tc, ExitStack() as ctx:
        if True:
            pool = lambda name, bufs, **kw: ctx.enter_context(
                tc.tile_pool(name=name, bufs=bufs, **kw))
            consts = pool("consts", 1)
            weights = pool("weights", 1)
            persist = pool("persist", 1)
            # PSUM pools: 8 banks total
            ps_mm = pool("ps_mm", 3, space="PSUM")
            ps_sc = pool("ps_sc", 3, space="PSUM")
            ps_ot = pool("ps_ot", 2, space="PSUM")
            p_x = pool("p_x", 1)
            p_xt = pool("p_xt", 1)
            p_qkt = pool("p_qkt", 1)
            p_v = pool("p_v", 2)
            p_exp = pool("p_exp", 3)
            p_out = pool("p_out", 2)
            p_small = pool("p_small", 2)
            p_cp = pool("p_cp", 2)
            p_wc = pool("p_wc", 2)
            p_dec = pool("p_dec", 2)

            # ---- constants & weights ----
            ident = consts.tile([128, 128], F32)
            nc.sync.dma_start(out=ident[:], in_=ident_src)
            maskt = consts.tile([128, 128], F32)
            nc.sync.dma_start(out=maskt[:], in_=maskt_src)

            wattn = weights.tile([128, 4, 3 * C], F32)
            nc.sync.dma_start(out=r(wattn[:]), in_=r(wattn_src))
            wproj = weights.tile([64, H, C], F32)
            nc.sync.dma_start(out=r(wproj[:]), in_=r(wproj_src))
            wproj2 = weights.tile([128, 4, C], F32)
            nc.sync.dma_start(out=r(wproj2[:]), in_=r(wproj2_src))

            lastT = persist.tile([64, H], F32)  # staging of out_last^T per batch

            # ================= attention phase (per local batch) ============
            for b in range(BPC):
                # load x_b [4 tok-chunks, 128, 512]
                x_sb = p_x.tile([128, 4, C], F32)
                nc.sync.dma_start(out=x_sb[:], in_=x_all[b])
                # transpose -> xT [128, cc, tok]
                xt_sb = p_xt.tile([128, 4, W], F32)
                for cc in range(4):
                    xp = ps_sc.tile([128, W], F32, tag="sc")
                    for t in range(4):
                        nc.tensor.transpose(
                            xp[:, t * 128:(t + 1) * 128],
                            x_sb[:, t, cc * 128:(cc + 1) * 128], ident[:])
                    nc.vector.tensor_copy(r(xt_sb[:, cc, :]), xp[:])

                # qkT [128, mc(8), tok] and v interleaved so head-0
                # operands (mc 0/4, v chunk 0) are ready earliest
                qkt = p_qkt.tile([128, 8, W], F32)
                v_sb = p_v.tile([128, 4, H, 65], F32)

                def qk_group(mc):
                    ps = ps_mm.tile([128, W], F32, tag="mm")
                    for kc in range(4):
                        nc.tensor.matmul(
                            ps[:],
                            r(wattn[:, kc, mc * 128:(mc + 1) * 128]),
                            r(xt_sb[:, kc, :]),
                            start=(kc == 0), stop=(kc == 3))
                    nc.vector.tensor_copy(r(qkt[:, mc, :]), ps[:])

                def v_group(t):
                    ps = ps_mm.tile([128, C], F32, tag="mm")
                    for kc in range(4):
                        nc.tensor.matmul(
                            ps[:],
                            r(xt_sb[:, kc, t * 128:(t + 1) * 128]),
                            r(wattn[:, kc, 2 * C:3 * C]),
                            start=(kc == 0), stop=(kc == 3))
                    nc.vector.tensor_copy(
                        r(v_sb[:, t, :, 0:64]),
                        ps[:].rearrange("p (h d) -> p h d", h=H))
                    nc.vector.memset(v_sb[:, t, :, 64], 1.0)

                qk_group(0); qk_group(4); v_group(0)
                qk_group(1); qk_group(5); v_group(1)
                qk_group(2); qk_group(6); v_group(2)
                qk_group(3); qk_group(7); v_group(3)

                outt = p_out.tile([64, H, W], F32)  # normalized outT per head

                def make_head(h, et, ot):
                    base = (h % 2) * 64
                    qt = qkt[base:base + 64, h // 2, :]
                    kt = qkt[base:base + 64, 4 + h // 2, :]

                    def scores_strip(ki):
                        n = W - ki * 128
                        sc = ps_sc.tile([128, W], F32, tag="sc")
                        nc.tensor.matmul(
                            sc[:, :n],
                            r(kt[:, ki * 128:(ki + 1) * 128]),
                            r(qt[:, ki * 128:]),
                            start=True, stop=True)
                        nc.scalar.activation(
                            r(et[:, ki, :n]), sc[:, :n], EXP, scale=0.125)
                        # causal 0/1 mask on the diagonal block
                        nc.vector.tensor_mul(
                            r(et[:, ki, :128]), r(et[:, ki, :128]), maskt[:])

                    def attnv_strip(ki):
                        n = W - ki * 128
                        nc.tensor.matmul(
                            ot[0:65, ki * 128:],
                            r(v_sb[:, ki, h, :]),
                            r(et[:, ki, :n]),
                            start=(ki == 0), stop=(ki == 3))

                    def finish():
                        recip = p_small.tile([1, W], F32)
                        nc.vector.reciprocal(recip[:], ot[64:65, :])
                        bcast = p_small.tile([64, W], F32)
                        nc.gpsimd.partition_broadcast(bcast[:], recip[:])
                        nc.vector.tensor_mul(
                            r(outt[:, h, :]), ot[0:64, :], bcast[:])

                    return scores_strip, attnv_strip, finish

                # heads in pairs: even head uses partitions 0-63 (PE rows
                # 0-63), odd head rows 64-127 -> score matmuls of the pair
                # land on disjoint PE row groups and can overlap.
                for hp in range(4):
                    h0, h1 = 2 * hp, 2 * hp + 1
                    et0 = p_exp.tile([128, 4, W], F32, tag="et")
                    ot0 = ps_ot.tile([65, W], F32, tag="ot")
                    et1 = p_exp.tile([128, 4, W], F32, tag="et")
                    ot1 = ps_ot.tile([65, W], F32, tag="ot")
                    s0, a0, f0 = make_head(h0, et0, ot0)
                    s1, a1, f1 = make_head(h1, et1, ot1)
                    s0(0); s1(0)
                    s0(1); a0(0)
                    s1(1); a1(0)
                    s0(2); a0(1)
                    s1(2); a1(1)
                    s0(3); a0(2)
                    s1(3); a1(2)
                    a0(3); a1(3)
                    f0(); f1()

                # stage out_last^T columns: lastT[d, h] = outT[d, h, 511]
                nc.vector.tensor_copy(lastT[:, :], outt[:, :, W - 1])

                # out proj: out[tok, :] = sum_h outT[:, h, tok].T @ Wproj[h]
                pr = p_cp.tile([128, 4, C], F32)
                for t in range(4):
                    ps = ps_mm.tile([128, C], F32, tag="mm")
                    for h in range(H):
                        nc.tensor.matmul(
                            ps[:],
                            r(outt[:, h, t * 128:(t + 1) * 128]),
                            r(wproj[:, h, :]),
                            start=(h == 0), stop=(h == 7))
                    nc.vector.tensor_copy(pr[:, t, :], ps[:])
                nc.sync.dma_start(out=out_all[b], in_=pr[:])

                # out_last natural row for this batch -> cc_in[b, h*64+d]
                cc_ap = cc_in[:]
                nc.sync.dma_start(
                    out=bass.AP(tensor=cc_ap.tensor, offset=b * C,
                                ap=[[1, 64], [64, H]]),
                    in_=lastT[:, :])

            # ================= collective =================
            nc.gpsimd.collective_compute(
                "AllGather",
                mybir.AluOpType.bypass,
                ins=[cc_in[:]],
                outs=[cc_out[:]],
                replica_groups=[list(range(N_CORES))],
            )

            # ================= decomposed phase =================
            ol = p_dec.tile([64, C], F32)  # out_last [64 batches, 512]
            nc.sync.dma_start(out=ol[:], in_=cc_out[:])
            lastT_all = persist.tile([128, 4, 64], BF16)
            for t in range(4):
                xp = ps_sc.tile([128, 64], F32, tag="sc")
                nc.tensor.transpose(
                    xp[:], ol[:, t * 128:(t + 1) * 128], ident[0:64, 0:64])
                nc.vector.tensor_copy(lastT_all[:, t, :], xp[:])

            for w in range(WPC):
                wc = p_wc.tile([128, 4, C], BF16)
                nc.scalar.dma_start(
                    out=wc[:], in_=wctx_all[:, :, w * C:(w + 1) * C])
                dps = ps_mm.tile([64, C], F32, tag="mm")
                for kc in range(4):
                    nc.tensor.matmul(
                        dps[0:64, :], lastT_all[:, kc, :], wc[:, kc, :],
                        start=(kc == 0), stop=(kc == 3))
                pv = p_dec.tile([64, C], F32)
                nc.sync.dma_start(out=pv[:], in_=prev_all[:, w, :])
                s_sb = p_dec.tile([64, C], F32)
                nc.vector.tensor_add(s_sb[:], dps[0:64, :], pv[:])
                st = p_dec.tile([128, 4, 64], F32)
                xp = ps_sc.tile([128, 256], F32, tag="sc")
                for t in range(4):
                    nc.tensor.transpose(
                        xp[:, t * 64:(t + 1) * 64],
                        s_sb[:, t * 128:(t + 1) * 128],
                        ident[0:64, 0:64])
                nc.scalar.copy(r(st[:]), xp[:])
                d2 = ps_sc.tile([64, C], F32, tag="sc")
                for t in range(4):
                    nc.tensor.matmul(
                        d2[0:64, :], r(st[:, t, :]), r(wproj2[:, t, :]),
                        start=(t == 0), stop=(t == 3))
                d2s = p_dec.tile([64, C], F32)
                nc.scalar.copy(d2s[:], d2[0:64, :])
                nc.sync.dma_start(out=dec_all[:, w, :], in_=d2s[:])

    nc.finalize()
    return nc


_NC_CACHE = None


def _get_nc():
    global _NC_CACHE
    if _NC_CACHE is None:
        _NC_CACHE = build_kernel()
    return _NC_CACHE


def make_in_maps(x, prev_decomposed, W_attn, W_ctx, W_proj, W_proj2):
    import ml_dtypes
    x = np.ascontiguousarray(x, dtype=np.float32)
    prev_decomposed = np.ascontiguousarray(prev_decomposed, dtype=np.float32)
    W_attn = np.ascontiguousarray(W_attn, dtype=np.float32)
    W_proj = np.ascontiguousarray(W_proj, dtype=np.float32)
    W_proj2 = np.ascontiguousarray(W_proj2, dtype=np.float32)
    W_ctx = np.ascontiguousarray(W_ctx).astype(ml_dtypes.bfloat16)
    ident = np.eye(128, dtype=np.float32)
    # scoresT layout [k, q]: keep k <= q within the diagonal block
    kk, qq = np.meshgrid(np.arange(128), np.arange(128), indexing="ij")
    maskt = np.where(kk > qq, np.float32(0.0), np.float32(1.0))

    in_maps = []
    for i in range(N_CORES):
        blob = np.empty(IN_WORDS, dtype=np.float32)
        blob[OX:OPREV] = x[i * BPC:(i + 1) * BPC].ravel()
        blob[OPREV:OWA] = prev_decomposed[:, i * WPC:(i + 1) * WPC, :].ravel()
        blob[OWA:OWP] = W_attn.ravel()
        blob[OWP:OWP2] = W_proj.ravel()
        blob[OWP2:OID] = W_proj2.ravel()
        blob[OID:OMK] = ident.ravel()
        blob[OMK:OWC] = maskt.ravel()
        wc = np.ascontiguousarray(W_ctx[:, i * WPC * C:(i + 1) * WPC * C])
        blob[OWC:IN_WORDS] = np.frombuffer(wc.tobytes(), dtype=np.float32)
        in_maps.append({"io_in": blob})
    return in_maps


def run(x, prev_decomposed, W_attn, W_ctx, W_proj, W_proj2, **spmd_kwargs):
    nc = _get_nc()
    in_maps = make_in_maps(x, prev_decomposed, W_attn, W_ctx, W_proj, W_proj2)
    res = run_bass_kernel_spmd(nc, in_maps, list(range(N_CORES)), **spmd_kwargs)
    results = res.results
    outs = []
    decs = []
    for i in range(N_CORES):
        blob = np.asarray(results[i]["io_out"])
        outs.append(blob[OOUT:ODEC].reshape(BPC, W, C))
        decs.append(blob[ODEC:OUT_WORDS].reshape(B, WPC, C))
    out = np.concatenate(outs, axis=0)
    dec = np.concatenate(decs, axis=1)
    return (out, dec), res


def kernel(x, prev_decomposed, W_attn, W_ctx, W_proj, W_proj2):
    (out, dec), _ = run(
        np.asarray(x, dtype=np.float32),
        np.asarray(prev_decomposed, dtype=np.float32),
        np.asarray(W_attn, dtype=np.float32),
        np.asarray(W_ctx, dtype=np.float32),
        np.asarray(W_proj, dtype=np.float32),
        np.asarray(W_proj2, dtype=np.float32))
    return (out, dec)


# revision 3
# speedup vs baseline: 5.1137x; 2.7942x over previous
"""Trainium2 Bass kernel for nn_MultiHeadAttention_47382079209593.

Full-input contract: kernel(**inputs) takes the complete unsharded tensors and
returns the full (out, decomposed) pair, distributing work across 8 NeuronCores
internally.

Sharding:
  - Attention (qkv proj, softmax, out proj): data-parallel over batch, 8
    batches per core.
  - decomposed = (out[:, -1, :] @ W_ctx): column-parallel over W_ctx's
    512*512 output dim -> core i owns block positions w in [64i, 64i+64) for
    ALL 64 batches.  The 64x512 last-token activations are shared via an
    on-device AllGather (16 KB per core).
  - decomposed2 = (prev + dec) @ W_proj2: row-parallel over the (b, w) dim,
    no communication needed.

All heavy matmuls run in float32r (full-rate fp32 PE mode, ~1.6e-4 rel err).

I/O packing: the PJRT-over-axon dispatch costs ~7 ms per executable argument
(measured: per-arg, not per-byte), so all logical inputs are packed into a
single 1-D f32 blob per core, and the output IS that same buffer: the XLA
custom call aliases output 0 to operand 0 (input-output donation), the
attention output overlays the x region (x[b] is fully consumed before out[b]
is written, both on the same DMA queue so FIFO order protects the overlay)
and dec overlays the prev region (prev[:,w] is read before dec[:,w] is
written, same queue). Net: ONE executable argument, ONE aliased output.
Offsets below are in f32 words; W_ctx is stored as bf16 bitcast into the f32
blob.
"""

import sys

if '/opt/trn_rl_repo' not in sys.path:
    sys.path.insert(0, '/opt/trn_rl_repo')

import numpy as np

import concourse.bass as bass
import concourse.tile as tile
from concourse import bacc, mybir

F32 = mybir.dt.float32
BF16 = mybir.dt.bfloat16
F32R = mybir.dt.float32r
EXP = mybir.ActivationFunctionType.Exp

B, W, C = 64, 512, 512
H = 8
DH = C // H          # 64
BLOCK = 512
N_CORES = 8
BPC = B // N_CORES   # 8 batches per core
WPC = BLOCK // N_CORES  # 64 block positions per core

# ---- packed I/O blob layout (f32 words, per core) ----
OX = 0                                  # x shard [BPC, W, C]; out overlays it
OPREV = OX + BPC * W * C                # prev shard [B, WPC, C]; dec overlays
OWA = OPREV + B * WPC * C               # W_attn       [C, 3C]
OWP = OWA + C * 3 * C                   # W_proj       [C, C]
OWP2 = OWP + C * C                      # W_proj2      [C, C]
OID = OWP2 + C * C                      # identity     [128, 128]
OMK = OID + 128 * 128                   # causal maskT [128, 128]
OWC = OMK + 128 * 128                   # W_ctx shard  [C, WPC*C] bf16
IO_WORDS = OWC + C * WPC * C // 2


def r(ap):
    return ap.bitcast(F32R)


def build_kernel():
    nc = bacc.Bacc("TRN2", num_devices=N_CORES)

    io_in = nc.dram_tensor("io_in", [IO_WORDS], F32, kind="ExternalInput")
    io_out = nc.dram_tensor("io_out", [IO_WORDS], F32, kind="ExternalOutput")

    x_all = io_in[OX:OPREV].rearrange("(b t p c) -> b p t c",
                                      b=BPC, t=4, p=128, c=C)
    prev_all = io_in[OPREV:OWA].rearrange("(b w c) -> b w c", b=B, w=WPC, c=C)
    wattn_src = io_in[OWA:OWP].rearrange("(k p c) -> p k c", k=4, p=128)
    wproj_src = io_in[OWP:OWP2].rearrange("(h p c) -> p h c", h=H, p=64)
    wproj2_src = io_in[OWP2:OID].rearrange("(k p c) -> p k c", k=4, p=128)
    ident_src = io_in[OID:OMK].rearrange("(p c) -> p c", p=128)
    maskt_src = io_in[OMK:OWC].rearrange("(p c) -> p c", p=128)
    wctx_all = io_in[OWC:IO_WORDS].bitcast(BF16).rearrange(
        "(k p m) -> p k m", k=4, p=128, m=WPC * C)

    out_all = io_out[OX:OPREV].rearrange("(b t p c) -> b p t c",
                                         b=BPC, t=4, p=128, c=C)
    dec_all = io_out[OPREV:OWA].rearrange("(b w c) -> b w c",
                                          b=B, w=WPC, c=C)

    cc_in = nc.dram_tensor("cc_in", [BPC, C], F32)
    cc_out = nc.dram_tensor("cc_out", [B, C], F32, addr_space="Shared")

    from contextlib import ExitStack

    with tile.TileContext(nc) as tc, ExitStack() as ctx:
        if True:
            pool = lambda name, bufs, **kw: ctx.enter_context(
                tc.tile_pool(name=name, bufs=bufs, **kw))
            consts = pool("consts", 1)
            weights = pool("weights", 1)
            persist = pool("persist", 1)
            # PSUM pools: 8 banks total
            ps_mm = pool("ps_mm", 3, space="PSUM")
            ps_sc = pool("ps_sc", 3, space="PSUM")
            ps_ot = pool("ps_ot", 2, space="PSUM")
            p_x = pool("p_x", 1)
            p_xt = pool("p_xt", 1)
            p_qkt = pool("p_qkt", 1)
            p_v = pool("p_v", 2)
            p_exp = pool("p_exp", 3)
            p_out = pool("p_out", 2)
            p_small = pool("p_small", 2)
            p_cp = pool("p_cp", 2)
            p_wc = pool("p_wc", 2)
            p_dec = pool("p_dec", 2)

            # ---- constants & weights ----
            ident = consts.tile([128, 128], F32)
            nc.sync.dma_start(out=ident[:], in_=ident_src)
            maskt = consts.tile([128, 128], F32)
            nc.sync.dma_start(out=maskt[:], in_=maskt_src)

            wattn = weights.tile([128, 4, 3 * C], F32)
            nc.sync.dma_start(out=r(wattn[:]), in_=r(wattn_src))
            wproj = weights.tile([64, H, C], F32)
            nc.sync.dma_start(out=r(wproj[:]), in_=r(wproj_src))
            wproj2 = weights.tile([128, 4, C], F32)
            nc.sync.dma_start(out=r(wproj2[:]), in_=r(wproj2_src))

            lastT = persist.tile([64, H], F32)  # staging of out_last^T per batch

            # ================= attention phase (per local batch) ============
            for b in range(BPC):
                # load x_b [4 tok-chunks, 128, 512]
                x_sb = p_x.tile([128, 4, C], F32)
                nc.sync.dma_start(out=x_sb[:], in_=x_all[b])
                # transpose -> xT [128, cc, tok]
                xt_sb = p_xt.tile([128, 4, W], F32)
                for cc in range(4):
                    xp = ps_sc.tile([128, W], F32, tag="sc")
                    for t in range(4):
                        nc.tensor.transpose(
                            xp[:, t * 128:(t + 1) * 128],
                            x_sb[:, t, cc * 128:(cc + 1) * 128], ident[:])
                    nc.vector.tensor_copy(r(xt_sb[:, cc, :]), xp[:])

                # qkT [128, mc(8), tok] and v interleaved so head-0
                # operands (mc 0/4, v chunk 0) are ready earliest
                qkt = p_qkt.tile([128, 8, W], F32)
                v_sb = p_v.tile([128, 4, H, 65], F32)

                def qk_group(mc):
                    ps = ps_mm.tile([128, W], F32, tag="mm")
                    for kc in range(4):
                        nc.tensor.matmul(
                            ps[:],
                            r(wattn[:, kc, mc * 128:(mc + 1) * 128]),
                            r(xt_sb[:, kc, :]),
                            start=(kc == 0), stop=(kc == 3))
                    nc.vector.tensor_copy(r(qkt[:, mc, :]), ps[:])

                def v_group(t):
                    ps = ps_mm.tile([128, C], F32, tag="mm")
                    for kc in range(4):
                        nc.tensor.matmul(
                            ps[:],
                            r(xt_sb[:, kc, t * 128:(t + 1) * 128]),
                            r(wattn[:, kc, 2 * C:3 * C]),
                            start=(kc == 0), stop=(kc == 3))
                    nc.vector.tensor_copy(
                        r(v_sb[:, t, :, 0:64]),
                        ps[:].rearrange("p (h d) -> p h d", h=H))
                    nc.vector.memset(v_sb[:, t, :, 64], 1.0)

                qk_group(0); qk_group(4); v_group(0)
                qk_group(1); qk_group(5); v_group(1)
                qk_group(2); qk_group(6); v_group(2)
                qk_group(3); qk_group(7); v_group(3)

                outt = p_out.tile([64, H, W], F32)  # normalized outT per head

                def make_head(h, et, ot):
                    base = (h % 2) * 64
                    qt = qkt[base:base + 64, h // 2, :]
                    kt = qkt[base:base + 64, 4 + h // 2, :]

                    def scores_strip(ki):
                        n = W - ki * 128
                        sc = ps_sc.tile([128, W], F32, tag="sc")
                        nc.tensor.matmul(
                            sc[:, :n],
                            r(kt[:, ki * 128:(ki + 1) * 128]),
                            r(qt[:, ki * 128:]),
                            start=True, stop=True)
                        nc.scalar.activation(
                            r(et[:, ki, :n]), sc[:, :n], EXP, scale=0.125)
                        # causal 0/1 mask on the diagonal block
                        nc.vector.tensor_mul(
                            r(et[:, ki, :128]), r(et[:, ki, :128]), maskt[:])

                    def attnv_strip(ki):
                        n = W - ki * 128
                        nc.tensor.matmul(
                            ot[0:65, ki * 128:],
                            r(v_sb[:, ki, h, :]),
                            r(et[:, ki, :n]),
                            start=(ki == 0), stop=(ki == 3))

                    def finish():
                        recip = p_small.tile([1, W], F32)
                        nc.vector.reciprocal(recip[:], ot[64:65, :])
                        bcast = p_small.tile([64, W], F32)
                        nc.gpsimd.partition_broadcast(bcast[:], recip[:])
                        nc.vector.tensor_mul(
                            r(outt[:, h, :]), ot[0:64, :], bcast[:])

                    return scores_strip, attnv_strip, finish

                # heads in pairs: even head uses partitions 0-63 (PE rows
                # 0-63), odd head rows 64-127 -> score matmuls of the pair
                # land on disjoint PE row groups and can overlap.
                for hp in range(4):
                    h0, h1 = 2 * hp, 2 * hp + 1
                    et0 = p_exp.tile([128, 4, W], F32, tag="et")
                    ot0 = ps_ot.tile([65, W], F32, tag="ot")
                    et1 = p_exp.tile([128, 4, W], F32, tag="et")
                    ot1 = ps_ot.tile([65, W], F32, tag="ot")
                    s0, a0, f0 = make_head(h0, et0, ot0)
                    s1, a1, f1 = make_head(h1, et1, ot1)
                    s0(0); s1(0)
                    s0(1); a0(0)
                    s1(1); a1(0)
                    s0(2); a0(1)
                    s1(2); a1(1)
                    s0(3); a0(2)
                    s1(3); a1(2)
                    a0(3); a1(3)
                    f0(); f1()

                # stage out_last^T columns: lastT[d, h] = outT[d, h, 511]
                nc.vector.tensor_copy(lastT[:, :], outt[:, :, W - 1])

                # out proj: out[tok, :] = sum_h outT[:, h, tok].T @ Wproj[h]
                pr = p_cp.tile([128, 4, C], F32)
                for t in range(4):
                    ps = ps_mm.tile([128, C], F32, tag="mm")
                    for h in range(H):
                        nc.tensor.matmul(
                            ps[:],
                            r(outt[:, h, t * 128:(t + 1) * 128]),
                            r(wproj[:, h, :]),
                            start=(h == 0), stop=(h == 7))
                    nc.vector.tensor_copy(pr[:, t, :], ps[:])
                nc.sync.dma_start(out=out_all[b], in_=pr[:])

                # out_last natural row for this batch -> cc_in[b, h*64+d]
                cc_ap = cc_in[:]
                nc.sync.dma_start(
                    out=bass.AP(tensor=cc_ap.tensor, offset=b * C,
                                ap=[[1, 64], [64, H]]),
                    in_=lastT[:, :])

            # ================= collective =================
            nc.gpsimd.collective_compute(
                "AllGather",
                mybir.AluOpType.bypass,
                ins=[cc_in[:]],
                outs=[cc_out[:]],
                replica_groups=[list(range(N_CORES))],
            )

            # ================= decomposed phase =================
            ol = p_dec.tile([64, C], F32)  # out_last [64 batches, 512]
            nc.sync.dma_start(out=ol[:], in_=cc_out[:])
            lastT_all = persist.tile([128, 4, 64], BF16)
            for t in range(4):
                xp = ps_sc.tile([128, 64], F32, tag="sc")
                nc.tensor.transpose(
                    xp[:], ol[:, t * 128:(t + 1) * 128], ident[0:64, 0:64])
                nc.vector.tensor_copy(lastT_all[:, t, :], xp[:])

            for w in range(WPC):
                wc = p_wc.tile([128, 4, C], BF16)
                nc.scalar.dma_start(
                    out=wc[:], in_=wctx_all[:, :, w * C:(w + 1) * C])
                dps = ps_mm.tile([64, C], F32, tag="mm")
                for kc in range(4):
                    nc.tensor.matmul(
                        dps[0:64, :], lastT_all[:, kc, :], wc[:, kc, :],
                        start=(kc == 0), stop=(kc == 3))
                pv = p_dec.tile([64, C], F32)
                nc.sync.dma_start(out=pv[:], in_=prev_all[:, w, :])
                s_sb = p_dec.tile([64, C], F32)
                nc.vector.tensor_add(s_sb[:], dps[0:64, :], pv[:])
                st = p_dec.tile([128, 4, 64], F32)
                xp = ps_sc.tile([128, 256], F32, tag="sc")
                for t in range(4):
                    nc.tensor.transpose(
                        xp[:, t * 64:(t + 1) * 64],
                        s_sb[:, t * 128:(t + 1) * 128],
                        ident[0:64, 0:64])
                nc.scalar.copy(r(st[:]), xp[:])
                d2 = ps_sc.tile([64, C], F32, tag="sc")
                for t in range(4):
                    nc.tensor.matmul(
                        d2[0:64, :], r(st[:, t, :]), r(wproj2[:, t, :]),
                        start=(t == 0), stop=(t == 3))
                d2s = p_dec.tile([64, C], F32)
                nc.scalar.copy(d2s[:], d2[0:64, :])
                nc.sync.dma_start(out=dec_all[:, w, :], in_=d2s[:])

    nc.finalize()
    return nc


_NC_CACHE = None
_EXEC_CACHE = None


def _get_nc():
    global _NC_CACHE
    if _NC_CACHE is None:
        _NC_CACHE = build_kernel()
    return _NC_CACHE


def build_exec():
    """Jitted 8-core SPMD executable: ONE donated operand per core whose
    buffer is aliased as the output (input-output donation via the custom
    call's lowering_input_output_aliases). Returns (sharded_fn, sharding)."""
    global _EXEC_CACHE
    if _EXEC_CACHE is not None:
        return _EXEC_CACHE
    import jax
    from jax.sharding import Mesh, PartitionSpec, NamedSharding
    from jax.experimental.shard_map import shard_map
    from concourse import bass2jax

    bass2jax.install_neuronx_cc_hook()
    nc = _get_nc()
    assert nc.dbg_addr is None
    assert nc.partition_id_tensor is not None
    part_name = nc.partition_id_tensor.name

    out_aval = jax.core.ShapedArray((IO_WORDS,), np.float32)

    def _body(blob):
        outs = bass2jax._bass_exec_p.bind(
            blob,
            bass2jax.partition_id_tensor(),
            out_avals=(out_aval,),
            in_names=("io_in", part_name),
            out_names=("io_out",),
            lowering_input_output_aliases=((0, 0),),
            sim_require_finite=True,
            sim_require_nnan=True,
            nc=nc,
        )
        return outs[0]

    devices = jax.devices()[:N_CORES]
    assert len(devices) == N_CORES
    mesh = Mesh(np.asarray(devices), ("core",))
    sharded = jax.jit(
        shard_map(_body, mesh=mesh, in_specs=(PartitionSpec("core"),),
                  out_specs=PartitionSpec("core"), check_rep=False),
        donate_argnums=(0,),
        keep_unused=True,
    )
    sharding = NamedSharding(mesh, PartitionSpec("core"))
    _EXEC_CACHE = (sharded, sharding)
    return _EXEC_CACHE


def make_blob(x, prev_decomposed, W_attn, W_ctx, W_proj, W_proj2):
    """Concatenated per-core packed input blobs, shape [N_CORES*IO_WORDS]."""
    import ml_dtypes
    x = np.ascontiguousarray(x, dtype=np.float32)
    prev_decomposed = np.ascontiguousarray(prev_decomposed, dtype=np.float32)
    W_attn = np.ascontiguousarray(W_attn, dtype=np.float32)
    W_proj = np.ascontiguousarray(W_proj, dtype=np.float32)
    W_proj2 = np.ascontiguousarray(W_proj2, dtype=np.float32)
    W_ctx = np.ascontiguousarray(W_ctx).astype(ml_dtypes.bfloat16)
    ident = np.eye(128, dtype=np.float32)
    # scoresT layout [k, q]: keep k <= q within the diagonal block
    kk, qq = np.meshgrid(np.arange(128), np.arange(128), indexing="ij")
    maskt = np.where(kk > qq, np.float32(0.0), np.float32(1.0))

    blob = np.empty(N_CORES * IO_WORDS, dtype=np.float32)
    for i in range(N_CORES):
        bl = blob[i * IO_WORDS:(i + 1) * IO_WORDS]
        bl[OX:OPREV] = x[i * BPC:(i + 1) * BPC].ravel()
        bl[OPREV:OWA] = prev_decomposed[:, i * WPC:(i + 1) * WPC, :].ravel()
        bl[OWA:OWP] = W_attn.ravel()
        bl[OWP:OWP2] = W_proj.ravel()
        bl[OWP2:OID] = W_proj2.ravel()
        bl[OID:OMK] = ident.ravel()
        bl[OMK:OWC] = maskt.ravel()
        wc = np.ascontiguousarray(W_ctx[:, i * WPC * C:(i + 1) * WPC * C])
        bl[OWC:IO_WORDS] = np.frombuffer(wc.tobytes(), dtype=np.float32)
    return blob


def unpack_blob(out_blob):
    """Full (out, dec) from the concatenated output blob."""
    out_blob = np.asarray(out_blob).reshape(N_CORES, IO_WORDS)
    outs = []
    decs = []
    for i in range(N_CORES):
        outs.append(out_blob[i, OX:OPREV].reshape(BPC, W, C))
        decs.append(out_blob[i, OPREV:OWA].reshape(B, WPC, C))
    return np.concatenate(outs, axis=0), np.concatenate(decs, axis=1)


def run(x, prev_decomposed, W_attn, W_ctx, W_proj, W_proj2):
    import jax
    sharded, sharding = build_exec()
    blob = make_blob(x, prev_decomposed, W_attn, W_ctx, W_proj, W_proj2)
    dev = jax.device_put(blob, sharding)
    dev.block_until_ready()
    out_blob = sharded(dev)
    out_blob.block_until_ready()
    return unpack_blob(out_blob)


def kernel(x, prev_decomposed, W_attn, W_ctx, W_proj, W_proj2):
    out, dec = run(
        np.asarray(x, dtype=np.float32),
        np.asarray(prev_decomposed, dtype=np.float32),
        np.asarray(W_attn, dtype=np.float32),
        np.asarray(W_ctx, dtype=np.float32),
        np.asarray(W_proj, dtype=np.float32),
        np.asarray(W_proj2, dtype=np.float32))
    return (out, dec)
